# revision 30
# baseline (speedup 1.0000x reference)
"""Trainium2 Bass kernel for nn_Lowpass: EMA recurrence over time.

level_t = (1-s) * x_t + s * level_{t-1},  s = sigmoid(smoothing_var)

Strategy (v5):
  - Data-parallel over batch: 16 batches -> 8 cores x 2 batches.
  - Time in blocks of L=128.  Within a block the recurrence is a
    lower-triangular matmul (stationary = roll(A,1)^T, fp16); the
    cross-block carry is an accumulating rank-1 matmul whose moving
    operand is the previous block's tail row (PSUM partition 0 after
    rotation -> legal moving-operand base partition).
  - fp16 everywhere off-chip except the final output: x is cast to
    fp16 on the host (halves load cost), block results are copied
    PSUM->SBUF as fp16 (DVE h0 / Act h1 -> two independent carry
    chains per batch), staged to an internal fp16 DRAM buffer with
    per-block DMAs (cheap: DRAM-side first dim is the row dim), and
    a final DRAM->DRAM gpsimd cast pass produces the f32 output at
    ~1.6us per 8-block chunk.
  - Stores un-rotate: rows 1..127 of the rotated tile go to y16 rows
    0..126 of the block; the tail rows (row 0 of each tile) are
    gathered in two batched DMAs per batch (blocks 0-7 / 8-15) so the
    cast chunks can fire as soon as their half of the batch is done.
"""

import os
import sys
import functools

sys.path.insert(0, "/opt/trn_rl_repo")
os.environ.setdefault("MYCRO_LOCAL_CACHE", "1")

import numpy as np

B, T, U = 16, 2048, 1024
NCORES = 8
BL = B // NCORES          # batches per core
L = 128                   # time block == partition dim
NBLK = T // L             # 16 blocks per batch
H = 512                   # matmul moving-free (one PSUM bank)

# ---- engine / queue assignment tables --------------------------------
# x loads: block0 solo (b0, b1), blocks 1-3 (b0, b1), then 4-block
# groups g=1..3 x (b0, b1)
LOADQ = ["sync", "gpsimd", "sync", "gpsimd",
         "sync", "gpsimd", "sync", "gpsimd", "sync", "gpsimd"]
# stage-1 block stores, order (k, b) for k=0..13: SP for b0, Pool for
# b1 except 4 b1-stores moved to SP (Pool carries the casts)
_B1Q = {1: "sync", 3: "sync", 5: "sync", 8: "sync"}
STOREQ = []
for _k in range(14):
    STOREQ += ["sync", _B1Q.get(_k, "gpsimd")]
STOREQ = tuple(STOREQ)
# tail stores (A-b0, A-b1, B-b0, B-b1)
TAILQ = ("sync", "gpsimd", "sync", "gpsimd")
# direct stores of blocks 14/15: rows14 (b0, b1), rows15 (b0, b1),
# tail rows 14 (b0, b1), tail rows 15 (b0, b1)
DIRQ = ("sync", "gpsimd", "sync", "sync",
        "scalar", "scalar", "scalar", "scalar")
# copies: per (k, b): (h0, h1) engines.  DVE h0, Act h1.
COPYE = tuple(("vector", "scalar") for _ in range(NBLK * BL))


@functools.lru_cache(maxsize=4)
def _build(loadq: tuple, storeq: tuple, tailq: tuple, dirq: tuple,
           copye: tuple, zero_c0: bool = True):
    import concourse.tile as tile
    from concourse import bacc, mybir

    nc = bacc.Bacc("TRN2", target_bir_lowering=False, debug=False)
    f32 = mybir.dt.float32
    f16 = mybir.dt.float16
    bf16 = mybir.dt.bfloat16
    x = nc.dram_tensor("x", [BL, T, U], f16, kind="ExternalInput").ap()
    at = nc.dram_tensor("at", [L, L], f16, kind="ExternalInput").ap()
    pc = nc.dram_tensor("pc", [1, L], f16, kind="ExternalInput").ap()
    pcb = nc.dram_tensor("pcb", [1, L], bf16, kind="ExternalInput").ap()
    c0 = nc.dram_tensor("c0", [1, U], f16, kind="ExternalInput").ap()
    # fp16 staging for blocks 0..13; y16[b, tau, k, :] = y[b, 128k+tau, :]
    NST = NBLK - 2
    y16 = nc.dram_tensor("y16", [BL, L, NST, U], f16, kind="Internal").ap()
    y = nc.dram_tensor("y", [BL, T, U], f32, kind="ExternalOutput").ap()

    Copy = mybir.ActivationFunctionType.Copy

    with tile.TileContext(nc) as tc:
        with (
            tc.tile_pool(name="const", bufs=1) as constp,
            tc.tile_pool(name="xin", bufs=1) as xinp,
            tc.tile_pool(name="ybuf", bufs=1) as ybufp,
            tc.tile_pool(name="ypsum", bufs=7, space="PSUM") as ypp,
            tc.tile_pool(name="warm", bufs=1, space="PSUM") as warmp,
        ):
            att = constp.tile([L, L], f16)
            pct = constp.tile([1, L], f16)
            pcbt = constp.tile([1, L], bf16)

            def new_yp():
                return ypp.tile([L, H], f32, name="yp")
            # PE p-state warm-up: ~3us of tiny serial matmuls so the real
            # matmuls run at max clock from the start.
            wsb = constp.tile([1, 256], f16)
            nc.vector.memset(wsb[:, :], 0.0)
            wps = warmp.tile([L, H], f32, name="warm")
            for _ in range(14):
                nc.tensor.matmul(wps[0:1, 0:256], lhsT=wsb[0:1, 0:1],
                                 rhs=wsb[0:1, 0:256], start=True, stop=True)
            nc.scalar.dma_start(pct[:, :], pc)
            nc.scalar.dma_start(pcbt[:, :], pcb)
            c0t = None
            if not zero_c0:
                c0t = constp.tile([1, U], f16)
                nc.scalar.dma_start(c0t[:, :], c0)

            # whole x shard upfront (fp16: 2KB/partition/block)
            xts = []
            for b in range(BL):
                xts.append(xinp.tile([L, NBLK * U], f16, name=f"xt{b}"))
            xvs = [xt[:, :].rearrange("p (k u) -> p k u", k=NBLK) for xt in xts]
            # block-0 first halves, then att, then second halves: the first
            # main matmul (h0) un-gates as early as possible.
            for b in range(BL):
                getattr(nc, loadq[b]).dma_start(
                    xvs[b][:, 0:1, 0:H],
                    x[b].rearrange("(k p) u -> p k u", p=L)[:, 0:1, 0:H],
                )
            nc.sync.dma_start(att[:, :], at)
            for b in range(BL):
                getattr(nc, loadq[b]).dma_start(
                    xvs[b][:, 0:1, H:U],
                    x[b].rearrange("(k p) u -> p k u", p=L)[:, 0:1, H:U],
                )
            nload = 2
            for b in range(BL):
                getattr(nc, loadq[nload]).dma_start(
                    xvs[b][:, 1:4, :],
                    x[b].rearrange("(k p) u -> p k u", p=L)[:, 1:4],
                )
                nload += 1
            for g in range(1, 4):
                for b in range(BL):
                    getattr(nc, loadq[nload]).dma_start(
                        xvs[b][:, 4 * g:4 * g + 4, :],
                        x[b].rearrange("(k p) u -> p k u", p=L)[:, 4 * g:4 * g + 4],
                    )
                    nload += 1

            ybs = []   # blocks 0..13 fp16 (rotated rows)
            yds = []   # blocks 14, 15 f32 (direct store)
            for b in range(BL):
                ybs.append(ybufp.tile([L, NST * U], f16, name=f"yb{b}"))
                yds.append(ybufp.tile([L, 2 * U], f32, name=f"yd{b}"))

            nstore = ncopy = ntail = 0
            for k in range(NBLK):
                for b in range(BL):
                    first = k == 0
                    e0, e1 = copye[ncopy]
                    ncopy += 1
                    late = k >= NST
                    for hh, eng in ((0, e0), (1, e1)):
                        sl = slice(hh * H, (hh + 1) * H)
                        yp = new_yp()
                        nc.tensor.matmul(
                            yp[:, :], lhsT=att[:, :], rhs=xvs[b][:, k, sl],
                            start=True, stop=(first and zero_c0),
                        )
                        if not (first and zero_c0):
                            if first:
                                lt, prevc = pct, c0t[0:1, sl]
                            elif k - 1 < NST:
                                off = (k - 1) * U
                                lt = pct
                                prevc = ybs[b][0:1, off + hh * H: off + (hh + 1) * H]
                            else:
                                # prev block lives in the f32 tile: read its
                                # row 0 as bf16 (high halves) to stay 1 c/row
                                off = (k - 1 - NST) * U
                                r = yds[b][0:1, off + hh * H: off + (hh + 1) * H]
                                lt = pcbt
                                prevc = r.bitcast(bf16).rearrange(
                                    "p (u two) -> p u two", two=2)[:, :, 1]
                            nc.tensor.matmul(
                                yp[:, :], lhsT=lt[:, :], rhs=prevc,
                                start=False, stop=True,
                            )
                        if late:
                            dst = yds[b][:, (k - NST) * U + hh * H:
                                         (k - NST) * U + (hh + 1) * H]
                        else:
                            dst = ybs[b][:, k * U + hh * H: k * U + (hh + 1) * H]
                        if eng == "scalar":
                            nc.scalar.activation(dst, yp[:, :], Copy)
                        else:
                            nc.vector.tensor_copy(dst, yp[:, :])
                    if late:
                        # direct f32 store: rows 1..127
                        getattr(nc, dirq[(k - NST) * BL + b]).dma_start(
                            y[b, k * L:k * L + L - 1, :],
                            yds[b][1:L, (k - NST) * U:(k - NST + 1) * U],
                        )
                    else:
                        getattr(nc, storeq[nstore]).dma_start(
                            y16[b, 0:127, k],
                            ybs[b][1:L, k * U:(k + 1) * U],
                        )
                        nstore += 1
                if k == 7 or k == NST - 1:
                    # tails -> y16[b, 127, k0:k+1, :]; then cast the chunk
                    k0 = 0 if k == 7 else 8
                    nk = k + 1 - k0
                    for b in range(BL):
                        getattr(nc, tailq[ntail]).dma_start(
                            y16[b, 127:128, k0:k + 1],
                            ybs[b][0:1, k0 * U:(k + 1) * U]
                            .rearrange("p (m u) -> p m u", m=nk),
                        )
                        ntail += 1
                    for b in range(BL):
                        nc.gpsimd.dma_start(
                            y[b, k0 * L:(k + 1) * L, :],
                            y16[b, :, k0:k + 1].rearrange("p k u -> k p u"),
                        )
                if k == NBLK - 2:
                    for b in range(BL):
                        # tail of block 14 -> y row 1919 (fires early)
                        getattr(nc, dirq[4 + b]).dma_start(
                            y[b, (NST + 1) * L - 1:(NST + 1) * L, :],
                            yds[b][0:1, 0:U],
                        )
                if k == NBLK - 1:
                    for b in range(BL):
                        # tail of block 15 -> y row 2047
                        getattr(nc, dirq[6 + b]).dma_start(
                            y[b, T - 1:T, :],
                            yds[b][0:1, U:2 * U],
                        )
    nc.compile()
    return nc


def _host_params(smoothing_var: np.ndarray):
    sm = smoothing_var.astype(np.float32).reshape(-1)
    return (1.0 / (1.0 + np.exp(-sm.astype(np.float64)))).astype(np.float32)


def _host_mats(s32_scalar):
    """Rotated stationary (fp16) and geometric column (fp16)."""
    s = np.float64(s32_scalar)
    j = np.arange(L)[:, None]
    i = np.arange(L)[None, :]
    A = np.where(j >= i, (1.0 - s) * s ** (j - i), 0.0)
    Arot = np.roll(A, 1, axis=0)          # PSUM row m = y[t0 + (m-1)%128]
    AT = np.ascontiguousarray(Arot.T.astype(np.float16))
    m = np.arange(L)
    pc64 = s ** (((m - 1) % L) + 1)
    pcol = pc64.astype(np.float16).reshape(1, L)
    import ml_dtypes
    pcolb = pc64.astype(ml_dtypes.bfloat16).reshape(1, L)
    return AT, pcol, pcolb


def kernel(inputs: np.ndarray, level_var: np.ndarray, smoothing_var: np.ndarray):
    from concourse import bass_utils

    x = np.ascontiguousarray(inputs, dtype=np.float32)
    assert x.shape == (B, T, U), x.shape
    s32 = _host_params(smoothing_var)
    if not np.all(s32 == s32[0]):
        return _kernel_numpy(x, level_var, s32)
    AT, pcol, pcolb = _host_mats(s32[0])
    c0 = np.ascontiguousarray(level_var.astype(np.float16).reshape(1, U))
    zero_c0 = bool(np.all(level_var == 0.0))
    x16 = x.astype(np.float16)

    nc = _build(tuple(LOADQ), STOREQ, TAILQ, DIRQ, COPYE, zero_c0)
    in_maps = [
        {"x": np.ascontiguousarray(x16[c * BL: (c + 1) * BL]), "at": AT,
         "pc": pcol, "pcb": pcolb, "c0": c0}
        for c in range(NCORES)
    ]
    res = bass_utils.run_bass_kernel_spmd(nc, in_maps, core_ids=list(range(NCORES)))
    out = np.concatenate([res.results[c]["y"] for c in range(NCORES)], axis=0)
    return out


def _kernel_numpy(x, level_var, s32):
    out = np.empty_like(x)
    c = np.broadcast_to(level_var.reshape(1, U), (x.shape[0], U)).astype(np.float32)
    one_minus = (1.0 - s32).astype(np.float32)
    for t in range(x.shape[1]):
        c = one_minus * x[:, t] + s32 * c
        out[:, t] = c
    return out


if __name__ == "__main__":
    rng = np.random.default_rng(0)
    xs = rng.standard_normal((B, T, U)).astype(np.float32)
    e = np.exp(-0.001 / 0.1)
    sm = np.full((1, U), np.log(e / (1 - e)), np.float32)
    lv = np.zeros((1, U), np.float32)
    o = kernel(xs, lv, sm)
    print("out", o.shape, o.dtype, float(np.abs(o).max()))


# revision 33
# speedup vs baseline: 1.0136x; 1.0136x over previous
"""Trainium2 Bass kernel for nn_Lowpass: EMA recurrence over time.

level_t = (1-s) * x_t + s * level_{t-1},  s = sigmoid(smoothing_var)

Strategy (v5):
  - Data-parallel over batch: 16 batches -> 8 cores x 2 batches.
  - Time in blocks of L=128.  Within a block the recurrence is a
    lower-triangular matmul (stationary = roll(A,1)^T, fp16); the
    cross-block carry is an accumulating rank-1 matmul whose moving
    operand is the previous block's tail row (PSUM partition 0 after
    rotation -> legal moving-operand base partition).
  - fp16 everywhere off-chip except the final output: x is cast to
    fp16 on the host (halves load cost), block results are copied
    PSUM->SBUF as fp16 (DVE h0 / Act h1 -> two independent carry
    chains per batch), staged to an internal fp16 DRAM buffer with
    per-block DMAs (cheap: DRAM-side first dim is the row dim), and
    a final DRAM->DRAM gpsimd cast pass produces the f32 output at
    ~1.6us per 8-block chunk.
  - Stores un-rotate: rows 1..127 of the rotated tile go to y16 rows
    0..126 of the block; the tail rows (row 0 of each tile) are
    gathered in two batched DMAs per batch (blocks 0-7 / 8-15) so the
    cast chunks can fire as soon as their half of the batch is done.
"""

import os
import sys
import functools

sys.path.insert(0, "/opt/trn_rl_repo")
os.environ.setdefault("MYCRO_LOCAL_CACHE", "1")

import numpy as np

B, T, U = 16, 2048, 1024
NCORES = 8
BL = B // NCORES          # batches per core
L = 128                   # time block == partition dim
NBLK = T // L             # 16 blocks per batch
H = 512                   # matmul moving-free (one PSUM bank)

# ---- engine / queue assignment tables --------------------------------
# x loads: block0 solo (b0, b1), blocks 1-3 (b0, b1), then 4-block
# groups g=1..3 x (b0, b1)
LOADQ = ["sync", "gpsimd", "sync", "gpsimd",
         "sync", "gpsimd", "sync", "gpsimd", "sync", "gpsimd"]
# stage-1 block stores, order (k, b) for k=0..13: SP for b0, Pool for
# b1 except 4 b1-stores moved to SP (Pool carries the casts)
_B1Q = {1: "sync", 3: "sync", 5: "sync", 8: "sync"}
STOREQ = []
for _k in range(14):
    STOREQ += ["sync", _B1Q.get(_k, "gpsimd")]
STOREQ = tuple(STOREQ)
# tail stores (A-b0, A-b1, B-b0, B-b1)
TAILQ = ("sync", "gpsimd", "sync", "scalar")
# direct stores of blocks 14/15: rows14 (b0, b1), rows15 (b0, b1),
# tail rows 14 (b0, b1), tail rows 15 (b0, b1)
DIRQ = ("sync", "gpsimd", "sync", "sync",
        "scalar", "scalar", "scalar", "scalar")
# copies: per (k, b): (h0, h1) engines.  DVE h0, Act h1.
COPYE = tuple(("vector", "scalar") for _ in range(NBLK * BL))


@functools.lru_cache(maxsize=4)
def _build(loadq: tuple, storeq: tuple, tailq: tuple, dirq: tuple,
           copye: tuple, zero_c0: bool = True):
    import concourse.tile as tile
    from concourse import bacc, mybir

    nc = bacc.Bacc("TRN2", target_bir_lowering=False, debug=False)
    f32 = mybir.dt.float32
    f16 = mybir.dt.float16
    bf16 = mybir.dt.bfloat16
    x = nc.dram_tensor("x", [BL, T, U], f16, kind="ExternalInput").ap()
    at = nc.dram_tensor("at", [L, L], f16, kind="ExternalInput").ap()
    pc = nc.dram_tensor("pc", [1, L], f16, kind="ExternalInput").ap()
    pcb = nc.dram_tensor("pcb", [1, L], bf16, kind="ExternalInput").ap()
    c0 = nc.dram_tensor("c0", [1, U], f16, kind="ExternalInput").ap()
    # fp16 staging for blocks 0..13; y16[b, tau, k, :] = y[b, 128k+tau, :]
    NST = NBLK - 2
    y16 = nc.dram_tensor("y16", [BL, L, NST, U], f16, kind="Internal").ap()
    y = nc.dram_tensor("y", [BL, T, U], f32, kind="ExternalOutput").ap()

    Copy = mybir.ActivationFunctionType.Copy

    with tile.TileContext(nc) as tc:
        with (
            tc.tile_pool(name="const", bufs=1) as constp,
            tc.tile_pool(name="xin", bufs=1) as xinp,
            tc.tile_pool(name="ybuf", bufs=1) as ybufp,
            tc.tile_pool(name="ypsum", bufs=7, space="PSUM") as ypp,
            tc.tile_pool(name="warm", bufs=1, space="PSUM") as warmp,
        ):
            att = constp.tile([L, L], f16)
            pct = constp.tile([1, L], f16)
            pcbt = constp.tile([1, L], bf16)

            def new_yp():
                return ypp.tile([L, H], f32, name="yp")
            # PE p-state warm-up: ~3us of tiny serial matmuls so the real
            # matmuls run at max clock from the start.
            wsb = constp.tile([1, 256], f16)
            nc.vector.memset(wsb[:, :], 0.0)
            wps = warmp.tile([L, H], f32, name="warm")
            for _ in range(14):
                nc.tensor.matmul(wps[0:1, 0:256], lhsT=wsb[0:1, 0:1],
                                 rhs=wsb[0:1, 0:256], start=True, stop=True)
            nc.scalar.dma_start(pct[:, :], pc)
            nc.scalar.dma_start(pcbt[:, :], pcb)
            c0t = None
            if not zero_c0:
                c0t = constp.tile([1, U], f16)
                nc.scalar.dma_start(c0t[:, :], c0)

            # whole x shard upfront (fp16: 2KB/partition/block)
            xts = []
            for b in range(BL):
                xts.append(xinp.tile([L, NBLK * U], f16, name=f"xt{b}"))
            xvs = [xt[:, :].rearrange("p (k u) -> p k u", k=NBLK) for xt in xts]
            # block-0 first halves, then att, then second halves: the first
            # main matmul (h0) un-gates as early as possible.
            for b in range(BL):
                getattr(nc, loadq[b]).dma_start(
                    xvs[b][:, 0:1, 0:H],
                    x[b].rearrange("(k p) u -> p k u", p=L)[:, 0:1, 0:H],
                )
            nc.sync.dma_start(att[:, :], at)
            for b in range(BL):
                getattr(nc, loadq[b]).dma_start(
                    xvs[b][:, 0:1, H:U],
                    x[b].rearrange("(k p) u -> p k u", p=L)[:, 0:1, H:U],
                )
            nload = 2
            for b in range(BL):
                getattr(nc, loadq[nload]).dma_start(
                    xvs[b][:, 1:4, :],
                    x[b].rearrange("(k p) u -> p k u", p=L)[:, 1:4],
                )
                nload += 1
            for g in range(1, 4):
                for b in range(BL):
                    getattr(nc, loadq[nload]).dma_start(
                        xvs[b][:, 4 * g:4 * g + 4, :],
                        x[b].rearrange("(k p) u -> p k u", p=L)[:, 4 * g:4 * g + 4],
                    )
                    nload += 1

            ybs = []   # blocks 0..13 fp16 (rotated rows)
            yds = []   # blocks 14, 15 f32 (direct store)
            for b in range(BL):
                ybs.append(ybufp.tile([L, NST * U], f16, name=f"yb{b}"))
                yds.append(ybufp.tile([L, 2 * U], f32, name=f"yd{b}"))

            nstore = ncopy = ntail = 0
            for k in range(NBLK):
                for b in range(BL):
                    first = k == 0
                    e0, e1 = copye[ncopy]
                    ncopy += 1
                    late = k >= NST
                    for hh, eng in ((0, e0), (1, e1)):
                        sl = slice(hh * H, (hh + 1) * H)
                        yp = new_yp()
                        nc.tensor.matmul(
                            yp[:, :], lhsT=att[:, :], rhs=xvs[b][:, k, sl],
                            start=True, stop=(first and zero_c0),
                        )
                        if not (first and zero_c0):
                            if first:
                                lt, prevc = pct, c0t[0:1, sl]
                            elif k - 1 < NST:
                                off = (k - 1) * U
                                lt = pct
                                prevc = ybs[b][0:1, off + hh * H: off + (hh + 1) * H]
                            else:
                                # prev block lives in the f32 tile: read its
                                # row 0 as bf16 (high halves) to stay 1 c/row
                                off = (k - 1 - NST) * U
                                r = yds[b][0:1, off + hh * H: off + (hh + 1) * H]
                                lt = pcbt
                                prevc = r.bitcast(bf16).rearrange(
                                    "p (u two) -> p u two", two=2)[:, :, 1]
                            nc.tensor.matmul(
                                yp[:, :], lhsT=lt[:, :], rhs=prevc,
                                start=False, stop=True,
                            )
                        if late:
                            dst = yds[b][:, (k - NST) * U + hh * H:
                                         (k - NST) * U + (hh + 1) * H]
                        else:
                            dst = ybs[b][:, k * U + hh * H: k * U + (hh + 1) * H]
                        if eng == "scalar":
                            nc.scalar.activation(dst, yp[:, :], Copy)
                        else:
                            nc.vector.tensor_copy(dst, yp[:, :])
                    if late:
                        # direct f32 store: rows 1..127
                        getattr(nc, dirq[(k - NST) * BL + b]).dma_start(
                            y[b, k * L:k * L + L - 1, :],
                            yds[b][1:L, (k - NST) * U:(k - NST + 1) * U],
                        )
                    else:
                        getattr(nc, storeq[nstore]).dma_start(
                            y16[b, 0:127, k],
                            ybs[b][1:L, k * U:(k + 1) * U],
                        )
                        nstore += 1
                if k == 7 or k == NST - 1:
                    # tails -> y16[b, 127, k0:k+1, :]; then cast the chunk
                    k0 = 0 if k == 7 else 8
                    nk = k + 1 - k0
                    for b in range(BL):
                        getattr(nc, tailq[ntail]).dma_start(
                            y16[b, 127:128, k0:k + 1],
                            ybs[b][0:1, k0 * U:(k + 1) * U]
                            .rearrange("p (m u) -> p m u", m=nk),
                        )
                        ntail += 1
                    for b in range(BL):
                        nc.gpsimd.dma_start(
                            y[b, k0 * L:(k + 1) * L, :],
                            y16[b, :, k0:k + 1].rearrange("p k u -> k p u"),
                        )
                if k == NBLK - 2:
                    for b in range(BL):
                        # tail of block 14 -> y row 1919 (fires early)
                        getattr(nc, dirq[4 + b]).dma_start(
                            y[b, (NST + 1) * L - 1:(NST + 1) * L, :],
                            yds[b][0:1, 0:U],
                        )
                if k == NBLK - 1:
                    for b in range(BL):
                        # tail of block 15 -> y row 2047
                        getattr(nc, dirq[6 + b]).dma_start(
                            y[b, T - 1:T, :],
                            yds[b][0:1, U:2 * U],
                        )
    nc.compile()
    return nc


def _host_params(smoothing_var: np.ndarray):
    sm = smoothing_var.astype(np.float32).reshape(-1)
    return (1.0 / (1.0 + np.exp(-sm.astype(np.float64)))).astype(np.float32)


def _host_mats(s32_scalar):
    """Rotated stationary (fp16) and geometric column (fp16)."""
    s = np.float64(s32_scalar)
    j = np.arange(L)[:, None]
    i = np.arange(L)[None, :]
    A = np.where(j >= i, (1.0 - s) * s ** (j - i), 0.0)
    Arot = np.roll(A, 1, axis=0)          # PSUM row m = y[t0 + (m-1)%128]
    AT = np.ascontiguousarray(Arot.T.astype(np.float16))
    m = np.arange(L)
    pc64 = s ** (((m - 1) % L) + 1)
    pcol = pc64.astype(np.float16).reshape(1, L)
    import ml_dtypes
    pcolb = pc64.astype(ml_dtypes.bfloat16).reshape(1, L)
    return AT, pcol, pcolb


def kernel(inputs: np.ndarray, level_var: np.ndarray, smoothing_var: np.ndarray):
    from concourse import bass_utils

    x = np.ascontiguousarray(inputs, dtype=np.float32)
    assert x.shape == (B, T, U), x.shape
    s32 = _host_params(smoothing_var)
    if not np.all(s32 == s32[0]):
        return _kernel_numpy(x, level_var, s32)
    AT, pcol, pcolb = _host_mats(s32[0])
    c0 = np.ascontiguousarray(level_var.astype(np.float16).reshape(1, U))
    zero_c0 = bool(np.all(level_var == 0.0))
    x16 = x.astype(np.float16)

    nc = _build(tuple(LOADQ), STOREQ, TAILQ, DIRQ, COPYE, zero_c0)
    in_maps = [
        {"x": np.ascontiguousarray(x16[c * BL: (c + 1) * BL]), "at": AT,
         "pc": pcol, "pcb": pcolb, "c0": c0}
        for c in range(NCORES)
    ]
    res = bass_utils.run_bass_kernel_spmd(nc, in_maps, core_ids=list(range(NCORES)))
    out = np.concatenate([res.results[c]["y"] for c in range(NCORES)], axis=0)
    return out


def _kernel_numpy(x, level_var, s32):
    out = np.empty_like(x)
    c = np.broadcast_to(level_var.reshape(1, U), (x.shape[0], U)).astype(np.float32)
    one_minus = (1.0 - s32).astype(np.float32)
    for t in range(x.shape[1]):
        c = one_minus * x[:, t] + s32 * c
        out[:, t] = c
    return out


if __name__ == "__main__":
    rng = np.random.default_rng(0)
    xs = rng.standard_normal((B, T, U)).astype(np.float32)
    e = np.exp(-0.001 / 0.1)
    sm = np.full((1, U), np.log(e / (1 - e)), np.float32)
    lv = np.zeros((1, U), np.float32)
    o = kernel(xs, lv, sm)
    print("out", o.shape, o.dtype, float(np.abs(o).max()))
